# revision 1
# baseline (speedup 1.0000x reference)
"""Trainium2 Bass kernel for nn_BakedAttentionHead.

Reference computation (per row b of query):
    s      = (q @ K^T) / sqrt(D)                      # (B, N)
    e'     = exp(s - max_n s)
    d      = 1 + sum_n e'
    recip  = 16-step sigmoid long-division approx of 1/d
    out    = (e' * recip) @ V

Kernel restructuring (algebraically equivalent, fp-wise ~1e-7 of reference):
    e      = exp(s)                 (raw; |s| <= ~6 so no overflow)
    em     = exp(-max_n s)
    d      = 1 + (sum_n e) * em
    out    = (e @ V) * (em * recip) per row

Sharding: data-parallel over the 8192 query rows -> 8 cores x 1024 rows,
keys/values replicated.  Matmuls run in float32r (full-rate fp32 PE mode).
mm1 computes scores^T ([n, m] orientation, 512 m per pair of output blocks)
so the exp'd tiles are directly the lhsT operand of mm2 with no transposes
of the big intermediate; only the tiny [128, 512] max/sum stat tensors go
through PE transposes for the cross-partition reduction.  The sigmoid
long-division scan runs between the mm2 compute and the output scale pass,
so neither the PE nor the PSUM-evacuating ACT queue ever waits on it.
"""

import numpy as np

B, D, N = 8192, 1024, 2048
NCORES = 8
M = B // NCORES            # 1024 query rows per core
NPAIR = 2                  # m "pairs" per core (one mm1 sweep each)
PW = M // NPAIR            # 512 m per pair = mm1 moving free dim
MT = PW // 128             # 4 output m-tiles of 128 rows per pair
NT = N // 128              # 16 n tiles
DT = D // 128              # 8 d (contraction) tiles
DO = 2                     # output dout chunks of 512
SCALE = 0.03125            # D ** -0.5
SIG_SCALE = 100.0
BITS = 16

_CACHE = {}


def _build(reps=1):
    import concourse.mybir as mybir
    import concourse.tile as tile
    from concourse import bacc
    from concourse.masks import make_identity
    from concourse.tile import add_dep_helper

    F32 = mybir.dt.float32
    F32R = mybir.dt.float32r
    AX = mybir.AxisListType
    OP = mybir.AluOpType
    AF = mybir.ActivationFunctionType

    nc = bacc.Bacc("TRN2", target_bir_lowering=False, debug=False,
                   num_devices=NCORES)
    qT_d = nc.declare_dram_parameter("qT", [D, M], F32R, isOutput=False)
    kT_d = nc.declare_dram_parameter("kT", [D, N], F32R, isOutput=False)
    v_d = nc.declare_dram_parameter("v", [N, D], F32R, isOutput=False)
    out_d = nc.declare_dram_parameter("out", [M, D], F32, isOutput=True)

    qT_ap = qT_d[:].rearrange("(dt p) m -> p dt m", p=128)
    kT_ap = kT_d[:].rearrange("(dt p) n -> p dt n", p=128)
    v_ap = v_d[:].rearrange("(nt p) do -> p nt do", p=128)

    with tile.TileContext(nc) as tc:
        with (
            tc.tile_pool(name="res", bufs=1) as res_pool,
            tc.tile_pool(name="e", bufs=NT) as e_pool,
            tc.tile_pool(name="acc", bufs=1) as acc_pool,
            tc.tile_pool(name="qt", bufs=1) as qt_pool,
            tc.tile_pool(name="stat", bufs=2) as stat_pool,
            tc.tile_pool(name="o", bufs=8) as out_pool,
            tc.tile_pool(name="ps1", bufs=3, space="PSUM") as ps1_pool,
            tc.tile_pool(name="ps2", bufs=5, space="PSUM") as ps2_pool,
        ):
            ident = res_pool.tile([128, 128], F32)
            make_identity(nc, ident[:])

            for rep in range(reps):
                # The SP HWDGE queue is FIFO and HBM bandwidth is one shared
                # pipe: emit loads in first-use order (kt + pair-0 qt gating
                # mm1 first, vt last -- it is only needed once mm2 starts).
                # qt streams per pair from a single slot: pair 1's load can
                # only start once pair 0's mm1 is done, which lands during
                # mm2(p0), well before mm1(p1) needs it.
                kt = res_pool.tile([128, DT, N], F32R)
                vt = res_pool.tile([128, NT, D], F32R)
                qts = []
                # fine-grained first loads: mm1's first matmuls need only the
                # first 128-n slice of kt plus qt0 dt-slices, so stage those
                # ahead of the bulk chunks to start the PE ~10 us earlier
                nc.sync.dma_start(out=kt[:, :, 0:128], in_=kT_ap[:, :, 0:128])
                qt0 = qt_pool.tile([128, DT, PW], F32R, name="qt0", tag="qt")
                for c in range(4):
                    nc.sync.dma_start(out=qt0[:, c * 2:(c + 1) * 2, :],
                                      in_=qT_ap[:, c * 2:(c + 1) * 2, 0:PW])
                qts.append(qt0)
                nc.sync.dma_start(out=kt[:, :, 128:512], in_=kT_ap[:, :, 128:512])
                for c in range(1, 4):
                    nc.sync.dma_start(out=kt[:, :, c * 512:(c + 1) * 512],
                                      in_=kT_ap[:, :, c * 512:(c + 1) * 512])
                for c in range(4):
                    nc.sync.dma_start(out=vt[:, c * 4:(c + 1) * 4, :],
                                      in_=v_ap[:, c * 4:(c + 1) * 4, :])
                qt1 = qt_pool.tile([128, DT, PW], F32R, name="qt1", tag="qt")
                nc.sync.dma_start(out=qt1[:], in_=qT_ap[:, :, PW:M])
                qts.append(qt1)

                def emit_mm1(p, st):
                    """scores^T for pair p: psum [128 n, 512 m] per n tile,
                    exp'd into f32r e tiles; elementwise max/sum accumulation
                    across n tiles; cross-partition stats via PE transpose."""
                    etiles = []
                    macc = acc_pool.tile([128, PW], F32, name=f"macc{p}",
                                         tag="macc")
                    sacc = acc_pool.tile([128, PW], F32, name=f"sacc{p}",
                                         tag="sacc")
                    for nt in range(NT):
                        ps = ps1_pool.tile([128, PW], F32, name=f"s{p}_{nt}",
                                           tag="ps1")
                        for dt in range(DT):
                            nc.tensor.matmul(
                                ps[:],
                                lhsT=kt[:, dt, nt * 128:(nt + 1) * 128],
                                rhs=qts[p][:, dt, :],
                                start=(dt == 0), stop=(dt == DT - 1),
                            )
                        e_nt = e_pool.tile([128, PW], F32R, name=f"e{p}_{nt}",
                                           tag="e")
                        nc.scalar.activation(e_nt[:], ps[:], AF.Exp, scale=SCALE)
                        if nt == 0:
                            nc.vector.tensor_copy(macc[:], ps[:])
                            nc.vector.tensor_copy(sacc[:], e_nt[:].bitcast(F32))
                        else:
                            nc.vector.tensor_tensor(
                                out=macc[:], in0=ps[:], in1=macc[:], op=OP.max)
                            nc.vector.tensor_tensor(
                                out=sacc[:], in0=e_nt[:].bitcast(F32),
                                in1=sacc[:], op=OP.add)
                        etiles.append(e_nt)
                    return etiles, macc, sacc

                def emit_stats(p, st, macc, sacc):
                    mx = stat_pool.tile([128, MT], F32, name=f"mx{p}", tag="mx")
                    sm = stat_pool.tile([128, MT], F32, name=f"sm{p}", tag="sm")
                    for c in range(MT):
                        pt = ps1_pool.tile([128, 128], F32, name=f"tm{p}_{c}",
                                           tag="ps1")
                        nc.tensor.transpose(
                            pt[:], macc[:, c * 128:(c + 1) * 128], ident[:])
                        nc.vector.tensor_reduce(
                            mx[:, c:c + 1], pt[:], axis=AX.X, op=OP.max)
                        pt2 = ps1_pool.tile([128, 128], F32, name=f"ts{p}_{c}",
                                            tag="ps1")
                        nc.tensor.transpose(
                            pt2[:], sacc[:, c * 128:(c + 1) * 128], ident[:])
                        nc.vector.tensor_reduce(
                            sm[:, c:c + 1], pt2[:], axis=AX.X, op=OP.add)
                    return mx, sm

                def emit_stats_d(p, st, mx, sm):
                    # em = exp(-scale*mx); d = 1 + sm*em.  Emitted after the
                    # first mm2 groups so no scan sigmoid is ready (d missing)
                    # while the early PSUM evacuations queue on ACT.
                    nc.scalar.activation(st["em"][:], mx[:], AF.Exp, scale=-SCALE)
                    tmp = stat_pool.tile([128, MT], F32, name=f"dt{p}",
                                         tag="dtmp")
                    nc.vector.tensor_tensor(out=tmp[:], in0=sm[:],
                                            in1=st["em"][:], op=OP.mult)
                    nc.vector.tensor_scalar_add(st["d"][:], tmp[:], 1.0)

                def emit_mm2(p, etiles, groups, otiles, evacs, st=None):
                    """out = e @ V, PSUM evacuated unscaled so the ACT queue
                    never waits on the reciprocal scan.  When st is given
                    (the pair's last groups, scan already done) the evac is
                    fused with the row scale on DVE, shortening the tail."""
                    for do, c in groups:
                            ps = ps2_pool.tile([128, 512], F32,
                                               name=f"o{p}_{do}_{c}", tag="ps2")
                            for nt in range(NT):
                                nc.tensor.matmul(
                                    ps[:],
                                    lhsT=etiles[nt][:, c * 128:(c + 1) * 128],
                                    rhs=vt[:, nt, do * 512:(do + 1) * 512],
                                    start=(nt == 0), stop=(nt == NT - 1),
                                )
                            ot = out_pool.tile([128, 512], F32,
                                               name=f"ot{p}_{do}_{c}", tag="ot")
                            if st is None:
                                ev = nc.scalar.activation(ot[:], ps[:], AF.Copy)
                                otiles[(do, c)] = ot
                                evacs.append(ev)
                            else:
                                nc.vector.tensor_scalar_mul(
                                    ot[:], ps[:], st["scale"][:, c:c + 1])
                                otiles[(do, c)] = None  # already scaled
                                m0 = p * PW + c * 128
                                nc.sync.dma_start(
                                    out=out_d[m0:m0 + 128,
                                              do * 512:(do + 1) * 512],
                                    in_=ot[:])

                def emit_scan(st, evacs):
                    """16-step sigmoid long-division on d: [128, 4] batched."""
                    d_t = st["d"]
                    r0 = stat_pool.tile([128, MT], F32, name="r0", tag="r0")
                    r1 = stat_pool.tile([128, MT], F32, name="r1", tag="r1")
                    q0 = stat_pool.tile([128, MT], F32, name="q0", tag="q0")
                    q1 = stat_pool.tile([128, MT], F32, name="q1", tag="q1")
                    z = stat_pool.tile([128, MT], F32, name="z", tag="z")
                    sg = stat_pool.tile([128, MT], F32, name="sg", tag="sg")
                    t = stat_pool.tile([128, MT], F32, name="t", tag="t")
                    nc.vector.memset(r0[:], 1.0)
                    nc.vector.memset(q0[:], 0.0)
                    r, qa = r0, q0
                    for i in range(BITS):
                        rn = r1 if r is r0 else r0
                        qn = q1 if qa is q0 else q0
                        nc.vector.scalar_tensor_tensor(      # z = 2r - d
                            out=z[:], in0=r[:], scalar=2.0, in1=d_t[:],
                            op0=OP.mult, op1=OP.subtract)
                        sig = nc.scalar.activation(          # step = sig(100 z)
                            sg[:], z[:], AF.Sigmoid, scale=SIG_SCALE)
                        if i >= 4:
                            # order-only hint: keep the slow sigmoid chain
                            # behind the PSUM evacuations in the ACT FIFO so
                            # mm2's psum recycling never waits on the scan
                            add_dep_helper(evacs[min(i - 4, len(evacs) - 1)].ins,
                                           sig.ins, True,
                                           "scan trails psum evacs")
                        nc.vector.tensor_tensor(             # t = d*step
                            out=t[:], in0=d_t[:], in1=sg[:], op=OP.mult)
                        nc.vector.scalar_tensor_tensor(      # r' = 2r - t
                            out=rn[:], in0=r[:], scalar=2.0, in1=t[:],
                            op0=OP.mult, op1=OP.subtract)
                        nc.vector.scalar_tensor_tensor(      # q' = w*step + q
                            out=qn[:], in0=sg[:], scalar=float(2.0 ** -(i + 1)),
                            in1=qa[:], op0=OP.mult, op1=OP.add)
                        r, qa = rn, qn
                    nc.vector.tensor_tensor(out=st["scale"][:], in0=st["em"][:],
                                            in1=qa[:], op=OP.mult)

                def emit_out(p, st, otiles):
                    """Apply the per-row scale and store."""
                    for do in range(DO):
                        for c in range(MT):
                            ot = otiles.get((do, c))
                            if ot is None:
                                continue
                            nc.vector.tensor_scalar_mul(
                                ot[:], ot[:], st["scale"][:, c:c + 1])
                            m0 = p * PW + c * 128
                            nc.sync.dma_start(
                                out=out_d[m0:m0 + 128, do * 512:(do + 1) * 512],
                                in_=ot[:])

                for p in range(NPAIR):
                    st = {
                        "em": stat_pool.tile([128, MT], F32, name=f"em{p}",
                                             tag="em"),
                        "d": stat_pool.tile([128, MT], F32, name=f"d{p}",
                                            tag="d"),
                        "scale": stat_pool.tile([128, MT], F32, name=f"sc{p}",
                                                tag="sc"),
                    }
                    etiles, macc, sacc = emit_mm1(p, st)
                    mx, sm = emit_stats(p, st, macc, sacc)
                    groups = [(do, c) for do in range(DO) for c in range(MT)]
                    otiles, evacs = {}, []
                    emit_mm2(p, etiles, groups[:2], otiles, evacs)
                    emit_stats_d(p, st, mx, sm)
                    emit_mm2(p, etiles, groups[2:6], otiles, evacs)
                    emit_scan(st, evacs)
                    emit_mm2(p, etiles, groups[6:], otiles, evacs, st=st)
                    emit_out(p, st, otiles)

    nc.compile()
    return nc


def _get_nc():
    if "nc" not in _CACHE:
        _CACHE["nc"] = _build()
    return _CACHE["nc"]


def kernel(query, keys, values):
    from concourse.bass_utils import run_bass_kernel_spmd

    query = np.ascontiguousarray(query, dtype=np.float32)
    keys = np.ascontiguousarray(keys, dtype=np.float32)
    values = np.ascontiguousarray(values, dtype=np.float32)

    nc = _get_nc()
    kT = np.ascontiguousarray(keys.T)
    in_maps = []
    for i in range(NCORES):
        qT = np.ascontiguousarray(query[i * M:(i + 1) * M].T)
        in_maps.append({"qT": qT, "kT": kT, "v": values})
    res = run_bass_kernel_spmd(nc, in_maps, list(range(NCORES)))
    out = np.concatenate([res.results[i]["out"] for i in range(NCORES)], axis=0)
    return np.ascontiguousarray(out, dtype=np.float32)



# revision 37
# speedup vs baseline: 1.2664x; 1.2664x over previous
"""Trainium2 Bass kernel for nn_BakedAttentionHead.

Reference computation (per row b of query):
    s      = (q @ K^T) / sqrt(D)                      # (B, N)
    e'     = exp(s - max_n s)
    d      = 1 + sum_n e'
    recip  = 16-step sigmoid long-division approx of 1/d
    out    = (e' * recip) @ V

Kernel restructuring (matches the reference to ~5e-3 of output absmax,
vs the 2e-2 gate):
    e      = ES * exp(s)            (ES=1/4 keeps e below fp8e4m3 max 240)
    emax   = max_n e  ( = ES * exp(max_n s), the "+1" of softmax1 in
                        unnormalized units )
    out    = (e @ V) * (1/(emax + sum_n e) - 2^-17/emax)
The reference's 16-step long-division reciprocal equals 1/d truncated to
16 fractional bits (plus sigmoid-soft-edge noise); an exact reciprocal
biased by half an ulp (the -2^-17 term) matches it to ~1.2e-3 of output
scale, measured on the real score distribution.

Precision scheme: every operand of both big matmuls is decomposed into
fp8e4m3 hi + lo parts (lo = fp8(x - fp8(x)), so hi+lo carries ~11
mantissa bits), and each matmul runs as three fp8 DoubleRow matmuls
(hi@hi + hi@lo + lo@hi; lo@lo is ~1e-6 relative and dropped).  DoubleRow
processes two 128-deep contraction planes per instruction at 0.5
cycles/row, so the three terms cost 0.75x the fp32r/bf16 cycles.  q/k/v
are split on the host (also halving input DMA); e hi/lo are made
on-chip: ACT evacuates each mm1 psum tile twice (exp -> f32 staging,
exp -> fp8 hi) and DVE subtracts for lo, so psum recycling depends on
ACT alone.  The row sum chain runs on GPSIMD and the max chain on DVE,
keeping every engine under the PE's tile period during mm1.

Schedule: both mm1 sweeps run back-to-back on the PE, then both mm2
sweeps.  Each pair's row-scale is ready ~2us after its mm1 ends (no
16-step scan), long before its mm2 evacuations, so every mm2 psum tile
is evacuated with the row scale fused into a single DVE op and DMA'd
straight out.  Input DMA interleaves k-hi/k-lo in 512-column chunks in
first-use order so the PE never waits past the pipeline fill.

Sharding: data-parallel over the 8192 query rows -> 8 cores x 1024 rows,
keys/values replicated.
"""

import math

import numpy as np

B, D, N = 8192, 1024, 2048
NCORES = 8
M = B // NCORES            # 1024 query rows per core
NPAIR = 2                  # m "pairs" per core (one mm1 sweep each)
PW = M // NPAIR            # 512 m per pair = mm1 moving free dim
MT = PW // 128             # 4 output m-tiles of 128 rows per pair
NT = N // 128              # 16 n tiles
DJ = D // 256              # 4 DoubleRow contraction steps for mm1
NJ = N // 256              # 8 DoubleRow contraction steps for mm2
SCALE = 0.03125            # D ** -0.5
ES = 0.25                  # e pre-scale: e = ES*exp(s) stays under fp8 max 240
LNES = math.log(ES)
QBIAS = 2.0 ** -17         # half-ulp of the reference's 16-bit long division

_CACHE = {}


def _build(reps=1):
    import concourse.mybir as mybir
    import concourse.tile as tile
    from concourse import bacc
    from concourse.masks import make_identity

    F32 = mybir.dt.float32
    F8 = mybir.dt.float8e4
    AX = mybir.AxisListType
    OP = mybir.AluOpType
    AF = mybir.ActivationFunctionType
    DR = mybir.MatmulPerfMode.DoubleRow

    nc = bacc.Bacc("TRN2", target_bir_lowering=False, debug=False,
                   num_devices=NCORES)
    qh_d = nc.declare_dram_parameter("qTh", [D, M], F8, isOutput=False)
    ql_d = nc.declare_dram_parameter("qTl", [D, M], F8, isOutput=False)
    kh_d = nc.declare_dram_parameter("kTh", [D, N], F8, isOutput=False)
    kl_d = nc.declare_dram_parameter("kTl", [D, N], F8, isOutput=False)
    vh_d = nc.declare_dram_parameter("vh", [N, D], F8, isOutput=False)
    vl_d = nc.declare_dram_parameter("vl", [N, D], F8, isOutput=False)
    out_d = nc.declare_dram_parameter("out", [M, D], F32, isOutput=True)

    qh_ap = qh_d[:].rearrange("(dt p) m -> p dt m", p=128)
    ql_ap = ql_d[:].rearrange("(dt p) m -> p dt m", p=128)
    kh_ap = kh_d[:].rearrange("(dt p) n -> p dt n", p=128)
    kl_ap = kl_d[:].rearrange("(dt p) n -> p dt n", p=128)
    vh_ap = vh_d[:].rearrange("(nt p) do -> p nt do", p=128)
    vl_ap = vl_d[:].rearrange("(nt p) do -> p nt do", p=128)

    with tile.TileContext(nc) as tc:
        with (
            tc.tile_pool(name="res", bufs=1) as res_pool,
            tc.tile_pool(name="e", bufs=2) as e_pool,
            tc.tile_pool(name="ef", bufs=4) as ef_pool,
            tc.tile_pool(name="acc", bufs=2) as acc_pool,
            tc.tile_pool(name="qt", bufs=2) as qt_pool,
            tc.tile_pool(name="stat", bufs=2) as stat_pool,
            tc.tile_pool(name="o", bufs=8) as out_pool,
            tc.tile_pool(name="ps1", bufs=5, space="PSUM") as ps1_pool,
            tc.tile_pool(name="ps2", bufs=3, space="PSUM") as ps2_pool,
        ):
            ident = res_pool.tile([128, 128], F32)
            make_identity(nc, ident[:])
            lnes = res_pool.tile([128, 1], F32)
            nc.vector.memset(lnes[:], LNES)

            for rep in range(reps):
                # SP HWDGE queue is FIFO: emit loads in first-use order.
                # mm1's first psum tile consumes kh[n0] + all qh, then kl[n0]
                # + all ql; later tiles consume kh/kl in n order, so those
                # stream as interleaved 512-column chunks.  v is only needed
                # once mm2 starts (~45us in).
                kh = res_pool.tile([128, DJ * 2, N], F8, name="kh", tag="kh")
                kl = res_pool.tile([128, DJ * 2, N], F8, name="kl", tag="kl")
                vh = res_pool.tile([128, NJ * 2, D], F8, name="vh", tag="vh")
                vl = res_pool.tile([128, NJ * 2, D], F8, name="vl", tag="vl")
                # pair-0 q rides the ACT HWDGE queue (idle until the first
                # exp) while k streams on the SP queue in parallel.  DMA
                # descriptors with contiguous runs under 512B move at half
                # rate, so k is chunked at exactly 512 columns.
                qts = []
                qt0h = qt_pool.tile([128, DJ * 2, PW], F8, name="q0h", tag="qh")
                qt0l = qt_pool.tile([128, DJ * 2, PW], F8, name="q0l", tag="ql")
                qt1h = qt_pool.tile([128, DJ * 2, PW], F8, name="q1h", tag="qh")
                qt1l = qt_pool.tile([128, DJ * 2, PW], F8, name="q1l", tag="ql")
                # DMA chain in consumption order.  Pair-0's q-lo rides AFTER
                # the first two k chunk-pairs: its matmuls (each tile's third
                # term) are deferred two tiles via interleaved open psum
                # groups, so the PE conveyor starts earlier.
                nc.sync.dma_start(out=kh[:, :, 0:128], in_=kh_ap[:, :, 0:128])
                nc.sync.dma_start(out=qt0h[:], in_=qh_ap[:, :, 0:PW])
                nc.sync.dma_start(out=kl[:, :, 0:128], in_=kl_ap[:, :, 0:128])
                nc.sync.dma_start(out=kh[:, :, 128:640],
                                  in_=kh_ap[:, :, 128:640])
                nc.sync.dma_start(out=kl[:, :, 128:640],
                                  in_=kl_ap[:, :, 128:640])
                nc.sync.dma_start(out=qt0l[:], in_=ql_ap[:, :, 0:PW])
                qts.append((qt0h, qt0l))
                for n0, n1 in [(640, 1280), (1280, 2048)]:
                    nc.sync.dma_start(out=kh[:, :, n0:n1],
                                      in_=kh_ap[:, :, n0:n1])
                    nc.sync.dma_start(out=kl[:, :, n0:n1],
                                      in_=kl_ap[:, :, n0:n1])
                nc.sync.dma_start(out=qt1h[:], in_=qh_ap[:, :, PW:M])
                nc.sync.dma_start(out=qt1l[:], in_=ql_ap[:, :, PW:M])
                qts.append((qt1h, qt1l))
                for c in range(0, NJ * 2, 4):
                    nc.sync.dma_start(out=vh[:, c:c + 4, :],
                                      in_=vh_ap[:, c:c + 4, :])
                    nc.sync.dma_start(out=vl[:, c:c + 4, :],
                                      in_=vl_ap[:, c:c + 4, :])

                warm_ps = [None]

                def emit_warm(n):
                    """Dummy identity matmuls: keep the PE busy while the
                    first input DMAs land so the p-state ramp (full clock
                    after 3us of continuous execution) is spent on throwaway
                    work and the real matmuls never pay it."""
                    if warm_ps[0] is None:
                        warm_ps[0] = ps2_pool.tile([128, 128], F32,
                                                   name="warm", tag="ps2")
                    for _ in range(n):
                        nc.tensor.matmul(warm_ps[0][:], lhsT=ident[:],
                                         rhs=ident[:], start=True, stop=True)

                def emit_mm1(p, defer=0, warm_sched=None, splice=None):
                    """scores^T for pair p ([n, m] orientation, 512 m), three
                    fp8 DoubleRow terms accumulated per psum tile; ACT
                    evacuates to e_f32 + e_hi(fp8), DVE makes e_lo, GPSIMD
                    accumulates the sum chain, DVE the max chain.

                    With defer=D, each tile's psum group is left open after
                    the two q-hi terms and the q-lo term lands D tiles later
                    (interleaved open psum groups), letting the PE conveyor
                    run while q-lo's DMA is still in flight."""
                    qth, qtl = qts[p]
                    warm_sched = warm_sched or {}
                    splice = splice or {}
                    e_hi = e_pool.tile([128, NT, PW], F8, name=f"eh{p}",
                                       tag="eh")
                    e_lo = e_pool.tile([128, NT, PW], F8, name=f"el{p}",
                                       tag="el")
                    macc = acc_pool.tile([128, PW], F32, name=f"macc{p}",
                                         tag="macc")
                    sacc = acc_pool.tile([128, PW], F32, name=f"sacc{p}",
                                         tag="sacc")
                    open_ps = {}

                    def a_pass(nt):
                        ps = ps1_pool.tile([128, PW], F32, name=f"s{p}_{nt}",
                                           tag="ps1")
                        i = 0
                        for kt_t in (kh, kl):
                            for j in range(DJ):
                                nc.tensor.matmul(
                                    ps[:],
                                    lhsT=kt_t[:, 2 * j:2 * j + 2,
                                              nt * 128:(nt + 1) * 128],
                                    rhs=qth[:, 2 * j:2 * j + 2, :],
                                    start=(i == 0), stop=False,
                                    perf_mode=DR,
                                )
                                i += 1
                        open_ps[nt] = ps

                    def b_pass(nt):
                        ps = open_ps.pop(nt)
                        for j in range(DJ):
                            nc.tensor.matmul(
                                ps[:],
                                lhsT=kh[:, 2 * j:2 * j + 2,
                                        nt * 128:(nt + 1) * 128],
                                rhs=qtl[:, 2 * j:2 * j + 2, :],
                                start=False, stop=(j == DJ - 1),
                                perf_mode=DR,
                            )
                        e_f = ef_pool.tile([128, PW], F32, name=f"ef{p}_{nt}",
                                           tag="ef")
                        nc.scalar.activation(e_f[:], ps[:], AF.Exp,
                                             scale=SCALE, bias=lnes[:, 0:1])
                        nc.scalar.activation(e_hi[:, nt, :], ps[:], AF.Exp,
                                             scale=SCALE, bias=lnes[:, 0:1])
                        nc.vector.tensor_tensor(
                            out=e_lo[:, nt, :], in0=e_f[:],
                            in1=e_hi[:, nt, :], op=OP.subtract)
                        if nt == 0:
                            nc.gpsimd.tensor_copy(sacc[:], e_f[:])
                            nc.vector.tensor_copy(macc[:], e_f[:])
                        else:
                            nc.gpsimd.tensor_tensor(
                                out=sacc[:], in0=e_f[:], in1=sacc[:],
                                op=OP.add)
                            nc.vector.tensor_tensor(
                                out=macc[:], in0=e_f[:], in1=macc[:],
                                op=OP.max)
                        if nt in splice:
                            splice[nt]()

                    for nt in range(NT):
                        if ("a", nt) in warm_sched:
                            emit_warm(warm_sched[("a", nt)])
                        a_pass(nt)
                        if nt >= defer:
                            if ("b", nt - defer) in warm_sched:
                                emit_warm(warm_sched[("b", nt - defer)])
                            b_pass(nt - defer)
                    for nt in range(NT - defer, NT):
                        b_pass(nt)
                    return e_hi, e_lo, macc, sacc

                def emit_stats(p, macc, sacc, scale_t):
                    """Cross-partition max/sum of the [128 n, 512 m] stat
                    accumulators via PE transposes, then the fused softmax1
                    scale: 1/(emax + sum) - 2^-17/emax."""
                    emax = stat_pool.tile([128, MT], F32, name=f"mx{p}",
                                          tag="mx")
                    sm = stat_pool.tile([128, MT], F32, name=f"sm{p}",
                                        tag="sm")
                    for c in range(MT):
                        pt = ps1_pool.tile([128, 128], F32, name=f"tm{p}_{c}",
                                           tag="ps1")
                        nc.tensor.transpose(
                            pt[:], macc[:, c * 128:(c + 1) * 128], ident[:])
                        nc.vector.tensor_reduce(
                            emax[:, c:c + 1], pt[:], axis=AX.X, op=OP.max)
                        pt2 = ps1_pool.tile([128, 128], F32, name=f"ts{p}_{c}",
                                            tag="ps1")
                        nc.tensor.transpose(
                            pt2[:], sacc[:, c * 128:(c + 1) * 128], ident[:])
                        nc.vector.tensor_reduce(
                            sm[:, c:c + 1], pt2[:], axis=AX.X, op=OP.add)
                    den = stat_pool.tile([128, MT], F32, name=f"den{p}",
                                         tag="den")
                    rmx = stat_pool.tile([128, MT], F32, name=f"rmx{p}",
                                         tag="rmx")
                    nc.vector.tensor_tensor(out=den[:], in0=emax[:],
                                            in1=sm[:], op=OP.add)
                    nc.vector.reciprocal(den[:], den[:])
                    nc.vector.reciprocal(rmx[:], emax[:])
                    nc.vector.scalar_tensor_tensor(
                        out=scale_t[:], in0=rmx[:], scalar=-QBIAS,
                        in1=den[:], op0=OP.mult, op1=OP.add)

                def emit_mm2(p, e_hi, e_lo, scale_t):
                    """out = e @ V as three fp8 DoubleRow terms per [128,
                    512] psum group (a psum bank holds 512 f32 per
                    partition), evacuated with the row scale fused on DVE
                    and DMA'd straight out.  The very last group of the last
                    pair is split in half so the final evac+store chain
                    after the PE finishes is as short as possible."""
                    chunks = [(c, do * 512, (do + 1) * 512)
                              for c in range(MT) for do in range(2)]
                    if p == NPAIR - 1:
                        c, d0, d1 = chunks.pop()
                        chunks += [(c, d0, d0 + 256), (c, d0 + 256, d1)]
                    for gi, (c, d0, d1) in enumerate(chunks):
                        ps = ps2_pool.tile([128, d1 - d0], F32,
                                           name=f"o{p}_{gi}", tag="ps2")
                        terms = [(e_hi, vh), (e_hi, vl), (e_lo, vh)]
                        nmm = len(terms) * NJ
                        i = 0
                        for e_t, v_t in terms:
                            for j in range(NJ):
                                nc.tensor.matmul(
                                    ps[:],
                                    lhsT=e_t[:, 2 * j:2 * j + 2,
                                             c * 128:(c + 1) * 128],
                                    rhs=v_t[:, 2 * j:2 * j + 2, d0:d1],
                                    start=(i == 0), stop=(i == nmm - 1),
                                    perf_mode=DR,
                                )
                                i += 1
                        ot = out_pool.tile([128, d1 - d0], F32,
                                           name=f"ot{p}_{gi}", tag="ot")
                        nc.vector.tensor_scalar_mul(
                            ot[:], ps[:], scale_t[:, c:c + 1])
                        m0 = p * PW + c * 128
                        nc.sync.dma_start(
                            out=out_d[m0:m0 + 128, d0:d1], in_=ot[:])

                scales = [stat_pool.tile([128, MT], F32, name=f"sc{p}",
                                         tag="sc") for p in range(NPAIR)]
                r0 = emit_mm1(0, defer=2,
                              warm_sched={("a", 0): 5, ("a", 1): 3,
                                          ("b", 0): 2})
                # stats(p0) transposes are spliced into mm1(p1)'s sweep so
                # the PE reaches them after pair 0's DVE/GPSIMD stat chains
                # have drained (no PE wait).
                r1 = emit_mm1(1, splice={2: lambda: emit_stats(
                    0, r0[2], r0[3], scales[0])})
                emit_mm2(0, r0[0], r0[1], scales[0])
                emit_stats(1, r1[2], r1[3], scales[1])
                emit_mm2(1, r1[0], r1[1], scales[1])

    nc.compile()
    return nc


def _get_nc():
    if "nc" not in _CACHE:
        _CACHE["nc"] = _build()
    return _CACHE["nc"]


def _split_fp8(x):
    import ml_dtypes

    f8 = ml_dtypes.float8_e4m3
    hi = x.astype(f8)
    lo = (x - hi.astype(np.float32)).astype(f8)
    return np.ascontiguousarray(hi), np.ascontiguousarray(lo)


def kernel(query, keys, values):
    from concourse.bass_utils import run_bass_kernel_spmd

    query = np.ascontiguousarray(query, dtype=np.float32)
    keys = np.ascontiguousarray(keys, dtype=np.float32)
    values = np.ascontiguousarray(values, dtype=np.float32)

    nc = _get_nc()
    kTh, kTl = _split_fp8(np.ascontiguousarray(keys.T))
    vh, vl = _split_fp8(values)
    in_maps = []
    for i in range(NCORES):
        qT = np.ascontiguousarray(query[i * M:(i + 1) * M].T)
        qTh, qTl = _split_fp8(qT)
        in_maps.append({"qTh": qTh, "qTl": qTl, "kTh": kTh, "kTl": kTl,
                        "vh": vh, "vl": vl})
    res = run_bass_kernel_spmd(nc, in_maps, list(range(NCORES)))
    out = np.concatenate([res.results[i]["out"] for i in range(NCORES)], axis=0)
    return np.ascontiguousarray(out, dtype=np.float32)
